# revision 52
# baseline (speedup 1.0000x reference)
"""CAM (channel attention) module kernel for Trainium2, 8 NeuronCores.

Reference computation (per batch b):
    q = x[b].reshape(C, N)                      # C=128, N=65536
    energy = q @ q.T                            # C x C
    att = softmax(rowmax(energy) - energy)      # == exp(rowmin(e)-e)/rowsum
    out = gamma * (att @ q) + x

Sharding: every core takes the same N/8 = 8192 column slice of BOTH
batches; the C x C energy partials are summed with one combined
AllReduce ([128,256] covering both batches -- the CC runtime has a
~10us fixed cost per op, so one op beats two).

Key schedule points:
  * Energy runs in single fp16 (PSUM accumulates fp32; products are
    O(100) so fp16 inputs cannot overflow).  The energy partials and
    the AllReduce stay fp32.
  * One combined [128,256] AllReduce covers both batches: the CC
    runtime costs ~10us fixed per op plus a start-of-NEFF barrier that
    rendezvouses all 8 cores, so minimizing op count wins.  Its staging
    DMAs run on the gpsimd queue, off the bulk load queue, so it
    triggers as soon as the partials are ready (~50us) rather than
    after all loads drain.
  * out = x is streamed to DRAM unconditionally while the AllReduce is
    in flight; the gamma*av term is then DMA-accumulated on top,
    predicated on the runtime value of gamma (BLAS beta==0 style).
    For the reference's gamma == 0 the accumulate DMAs skip themselves,
    so the post-attention tail is only PSUM-copy bound and no store
    traffic remains after the collective.  Both paths are correct for
    any gamma.
"""

import numpy as np

import concourse.bass as bass
import concourse.mybir as mybir
import concourse.tile as tile
from concourse import bacc
from concourse.bass_utils import run_bass_kernel_spmd
from concourse.masks import make_identity

B, C, D, H, W = 2, 128, 16, 64, 64
N = D * H * W  # 65536
NCORES = 8
NS = N // NCORES  # 8192 columns per core per batch

F32 = mybir.dt.float32
F16 = mybir.dt.float16
I32 = mybir.dt.int32

# tuning knobs
CFG = dict(
    nb=2048,          # cast granularity
    load_chunks=(512, 512, 1024, 1024, 1024, 1024, 1024, 1024, 1024),
    store_nb=2048,    # output store DMA granularity
    avf=512,          # AV matmul free-dim chunk (one psum bank)
    av_bufs=4,
    use_collective=True,
    cond_stores=True,
)

GROUPS = [[0, 1, 2, 3, 4, 5, 6, 7]]


def _body(nc: bass.Bass, tc: "tile.TileContext", xs, gm, out, cfg):
    NB = cfg["nb"]
    AVF = cfg["avf"]
    JCH = NS // 128          # transposed 128-chunks per batch
    GB = 512                 # transpose group (one psum tile)
    gjp = GB // 128          # chunks per transpose group
    with (
        tc.tile_pool(name="big", bufs=1) as big,
        tc.tile_pool(name="small", bufs=1) as small,
        tc.tile_pool(name="work", bufs=3) as work,
        tc.tile_pool(name="psum_e", bufs=1, space="PSUM") as pse,
        tc.tile_pool(name="psum_av", bufs=cfg["av_bufs"], space="PSUM") as psav,
        tc.tile_pool(name="trps", bufs=2, space="PSUM") as trps,
        tc.tile_pool(name="dram", bufs=1, space="DRAM") as dram,
    ):
        # Persistent SBUF tensors; column range [b*NS, (b+1)*NS) = batch b
        xf = big.tile([C, 2 * NS], F32, tag="xf")      # exact f32 x
        qh = big.tile([C, 2 * NS], F16, tag="qh")      # fp16 cast
        qT = big.tile([128, 2 * JCH, 128], F16, tag="qT")  # transposed chunks

        identh = small.tile([128, 128], F16, tag="identh")
        make_identity(nc, identh)

        g0 = small.tile([1, 1], F32, tag="g0")
        gsb = small.tile([128, 1], F32, tag="gsb")
        nc.sync.dma_start(g0[:], gm[None, :])
        nc.gpsimd.partition_broadcast(gsb, g0[:])

        # "gamma != 0" predicate on the accumulate-store engine (gpsimd is
        # the only engine whose software DGE supports dma accum)
        conds = {}
        if cfg["cond_stores"]:
            g0i = g0[:, :].bitcast(I32)
            gv = nc.gpsimd.value_load(g0i)
            conds[nc.gpsimd] = gv != 0

        e_out = nc.dram_tensor("e_out", [128, 256], F32, addr_space="Shared")
        e_sb = small.tile([128, 256], F32, tag="e_sb")

        ec_ps = [
            pse.tile([128, 128], F32, tag=f"ec_ps{b}", name=f"ec_ps{b}")
            for b in range(2)
        ]

        def load(b):
            pos = b * NS
            engs = [nc.sync, nc.scalar]
            for i, ln in enumerate(cfg["load_chunks"]):
                engs[i % 2].dma_start(xf[:, pos:pos + ln], xs[:, pos:pos + ln])
                pos += ln
            assert pos == (b + 1) * NS

        def early_stores(b):
            """Unconditionally stream out = x while the AR is in flight;
            when gamma != 0 the gamma*av term is DMA-accumulated on top
            later, when gamma == 0 (the common CAM init) this IS the
            final output and the accumulate pass skips itself."""
            SNB = cfg["store_nb"]
            for i in range(NS // SNB):
                lo = b * NS + i * SNB
                nc.sync.dma_start(out[:, lo:lo + SNB], xf[:, lo:lo + SNB])

        def phase1(b):
            """cast -> PE-transpose -> energy MMs for batch b."""
            base = b * NS
            jbase = b * JCH

            def emit_emm(jlist):
                for j in jlist:
                    jj = jbase + j
                    nc.tensor.matmul(
                        ec_ps[b], lhsT=qT[:, jj, :], rhs=qT[:, jj, :],
                        start=(j == 0), stop=(j == JCH - 1),
                    )

            # small first cast blocks so the PE starts as soon as the
            # first load chunk lands, larger ones for steady state
            cast_plan = (512, 512, 1024) + (NB,) * ((NS - 2048) // NB)
            assert sum(cast_plan) == NS
            pos = 0
            g = 0
            for cb in cast_plan:
                sl = slice(base + pos, base + pos + cb)
                nc.vector.tensor_copy(qh[:, sl], xf[:, sl])        # fp16 cast
                for gg in range(cb // GB):
                    th = trps.tile([128, GB], F16, tag="th")
                    for u in range(gjp):
                        a0 = base + pos + gg * GB + u * 128
                        ps = slice(u * 128, (u + 1) * 128)
                        nc.tensor.transpose(th[:, ps], qh[:, a0:a0 + 128], identh)
                    jsl = slice(jbase + g * gjp, jbase + (g + 1) * gjp)
                    nc.scalar.copy(
                        qT[:, jsl, :],
                        th.rearrange("p (a b) -> p a b", b=128),
                    )
                    if g > 0:
                        emit_emm(range((g - 1) * gjp, g * gjp))
                    g += 1
                pos += cb
            emit_emm(range(JCH - gjp, JCH))

        def stage_energy(b):
            """Copy batch b's energy partial into the combined staging tile."""
            nc.vector.tensor_copy(e_sb[:, b * 128:(b + 1) * 128], ec_ps[b])

        def reduce_energy():
            """One combined AllReduce for both batches' 128x128 partials.

            The CC runtime costs ~10us fixed per op, so a single
            [128,256] AllReduce beats two serial 64KB ones.
            """
            if not cfg["use_collective"]:
                return e_sb
            e_in = dram.tile([128, 256], F32, tag="e_in")
            nc.gpsimd.dma_start(e_in[:], e_sb)
            nc.gpsimd.collective_compute(
                "AllReduce",
                mybir.AluOpType.add,
                replica_groups=GROUPS,
                ins=[e_in.opt()],
                outs=[e_out.ap()[:, :].opt()],
                unique_tensors="Yes",
            )
            # fetch in halves: batch 0's softmax starts on its half alone
            e_full = small.tile([128, 256], F32, tag="e_full")
            nc.gpsimd.dma_start(e_full[:, 0:128], e_out.ap()[:, 0:128])
            nc.gpsimd.dma_start(e_full[:, 128:256], e_out.ap()[:, 128:256])
            return e_full

        def softmax_attT(b, e_full):
            """Unnormalized att^T (fp16) + per-row scale gamma/rowsum.

            The softmax normalization and the gamma factor are folded
            into the per-row scale applied at the AV PSUM copies, which
            shortens the post-AllReduce critical chain by two DVE hops.
            """
            e_b = e_full[:, b * 128:(b + 1) * 128]
            m = small.tile([128, 1], F32, tag=f"m{b}")
            nc.vector.tensor_reduce(
                m, e_b, axis=mybir.AxisListType.X, op=mybir.AluOpType.min
            )
            att = small.tile([128, 128], F16, tag=f"att{b}")
            r = small.tile([128, 1], F32, tag=f"r{b}")
            nc.scalar.activation(
                att, e_b, mybir.ActivationFunctionType.Exp,
                bias=m, scale=-1.0, accum_out=r,
            )
            if b == 0:
                # PE clock warmers gated on att: ramp the tensor engine
                # out of its idle p-state while the softmax chain runs,
                # so the first real AV matmuls start near full speed
                for i in range(8):
                    wps = trps.tile([128, 128], F16, tag="th", name=f"warm{i}")
                    nc.tensor.transpose(wps, att, identh)
            attT_ps = trps.tile([128, 128], F16, tag="th", name=f"attT_ps{b}")
            nc.tensor.transpose(attT_ps, att, identh)
            attT = small.tile([128, 128], F16, tag=f"attT{b}")
            nc.scalar.copy(attT, attT_ps)
            # off the critical path: gr = gamma / rowsum
            rinv = small.tile([128, 1], F32, tag=f"rinv{b}")
            nc.vector.reciprocal(rinv, r)
            gr = small.tile([128, 1], F32, tag=f"gr{b}")
            nc.vector.tensor_scalar(
                gr, rinv, gsb, None, mybir.AluOpType.mult
            )
            return attT, gr

        def av_phase(attTs, grs):
            """Interleaved AV matmuls for both batches + accumulate-store.

            Both attention matrices are ready when the combined AR lands,
            so the batches interleave.  The residual is handled by the
            early x stores: here each gamma*av chunk is only copied off
            PSUM (DVE/scalar alternating) and then DMA-accumulated onto
            out, predicated on gamma != 0 -- for gamma == 0 the
            accumulate pass skips itself and the tail is copy-bound.
            """
            SNB = cfg["store_nb"]
            per_store = SNB // AVF
            nchunks = NS // AVF
            o_sbs = [None, None]
            def get_av_ps(f):
                # 8-deep PSUM rotation: 4 dedicated banks plus the energy
                # and transpose banks, which are free once the AllReduce
                # is staged -- keeps the PE matmul stream from stalling
                # on copy-paced recycling (stalls reset the p-state ramp)
                r = f % 8
                if r < 4:
                    return psav.tile([128, AVF], F32, tag="av_ps",
                                     name=f"avps{f}")
                if r < 6:
                    return pse.tile([128, AVF], F32, tag=f"ec_ps{r - 4}",
                                    name=f"avps{f}")
                return trps.tile([128, AVF], F32, tag="th", name=f"avps{f}")

            for f in range(2 * nchunks):
                b, c = f % 2, f // 2
                base = b * NS
                sl = slice(base + c * AVF, base + (c + 1) * AVF)
                av_ps = get_av_ps(f)
                nc.tensor.matmul(av_ps, lhsT=attTs[b], rhs=qh[:, sl],
                                 start=True, stop=True)
                if c % per_store == 0:
                    o_sbs[b] = work.tile([128, SNB], F32, tag=f"o_sb{b}",
                                         name=f"o_sb{b}_{c}")
                o_sb = o_sbs[b]
                osl = slice((c % per_store) * AVF, (c % per_store + 1) * AVF)
                if f % 2 == 0:
                    nc.vector.tensor_scalar(
                        o_sb[:, osl], av_ps, grs[b], None, mybir.AluOpType.mult
                    )
                else:
                    nc.scalar.activation(
                        o_sb[:, osl], av_ps,
                        mybir.ActivationFunctionType.Copy, scale=grs[b],
                    )
                if (c + 1) % per_store == 0:
                    lo = base + (c + 1 - per_store) * AVF
                    hi = base + (c + 1) * AVF
                    cond = conds[nc.gpsimd] if cfg["cond_stores"] else None
                    nc.gpsimd.dma_start(out[:, lo:hi], o_sb, cond=cond,
                                        accum_op=mybir.AluOpType.add)

        # ---- pipelined schedule over the two batches ----
        load(0)
        load(1)
        early_stores(0)
        early_stores(1)
        phase1(0)
        stage_energy(0)
        phase1(1)
        stage_energy(1)
        ef = reduce_energy()
        a0, g0r = softmax_attT(0, ef)
        a1, g1r = softmax_attT(1, ef)
        av_phase([a0, a1], [g0r, g1r])


_cached_nc = None


def _build(cfg=None):
    cfg = dict(CFG, **(cfg or {}))
    nc = bacc.Bacc(
        "TRN2",
        target_bir_lowering=False,
        debug=False,
        enable_asserts=False,
        num_devices=NCORES,
    )
    xs = nc.dram_tensor("xs", [C, 2 * NS], F32, kind="ExternalInput").ap()
    gm = nc.dram_tensor("gamma", [1], F32, kind="ExternalInput").ap()
    out = nc.dram_tensor("out", [C, 2 * NS], F32, kind="ExternalOutput").ap()
    with tile.TileContext(nc) as tc:
        _body(nc, tc, xs, gm, out, cfg)
    nc.compile()
    return nc


def kernel(x: np.ndarray, gamma: np.ndarray, _collect_results=None) -> np.ndarray:
    global _cached_nc
    if _cached_nc is None:
        _cached_nc = _build()
    nc = _cached_nc

    xr = np.ascontiguousarray(np.asarray(x, dtype=np.float32).reshape(B, C, N))
    gamma = np.ascontiguousarray(np.asarray(gamma, dtype=np.float32))
    in_maps = []
    for k in range(NCORES):
        shard = np.concatenate(
            [xr[0, :, k * NS:(k + 1) * NS], xr[1, :, k * NS:(k + 1) * NS]],
            axis=1,
        )
        in_maps.append({"xs": np.ascontiguousarray(shard), "gamma": gamma})

    res = run_bass_kernel_spmd(nc, in_maps, core_ids=list(range(NCORES)))
    if _collect_results is not None:
        _collect_results.append(res)

    outf = np.empty((B, C, N), np.float32)
    for k in range(NCORES):
        o = res.results[k]["out"]
        outf[0, :, k * NS:(k + 1) * NS] = o[:, :NS]
        outf[1, :, k * NS:(k + 1) * NS] = o[:, NS:]
    return outf.reshape(B, C, D, H, W)
